# revision 1
# baseline (speedup 1.0000x reference)
"""ViT-Base forward + JVP on 8 trn2 NeuronCores, data-parallel over batch.

Activations: [d_model partition-tiles of 128, tokens], fp32 residual, fp16
matmul operands, fp32 PSUM. Per-token stats via ones-matmuls; applied via PE
outer-product broadcasts. Attention in transposed-scores layout [s_k, s_q];
V produced directly in token-partition layout, so no transposes anywhere.
"""

import os
import sys

sys.path.insert(0, "/opt/trn_rl_repo")

import numpy as np

from concourse import bacc, bass, mybir, tile

F32 = mybir.dt.float32
F16 = mybir.dt.float16
AF = mybir.ActivationFunctionType
OP = mybir.AluOpType

IMG, P, L, HEADS, D, MLPD, NCLS, B = 224, 16, 12, 12, 768, 3072, 1000, 32
G = IMG // P
S = G * G + 1            # 197
DH = D // HEADS          # 64
EPS = 1e-6
NCORES = 8
NI = B // NCORES         # 4
T = NI * S               # 788
KT = D // 128            # 6
KM = MLPD // 128         # 24
NCH = 2
TC0 = T // NCH           # 394
L_RUN = int(os.environ.get("VIT_LAYERS", "12"))


def _f16(a):
    return np.ascontiguousarray(a, dtype=np.float16)


def _f32(a):
    return np.ascontiguousarray(a, dtype=np.float32)


def prep_host(inp):
    d = {}
    d["wconv"] = _f16(inp["conv_w"].reshape(D, 3 * P * P).T)
    d["wqk"] = _f16(np.transpose(inp["qkv_w"][:, : 2 * D, :], (0, 2, 1)))
    d["wv"] = _f16(np.transpose(inp["qkv_w"][:, 2 * D :, :], (0, 2, 1)))
    d["wo"] = _f16(np.transpose(inp["out_w"], (0, 2, 1)))
    d["w1"] = _f16(np.transpose(inp["fc1_w"], (0, 2, 1)))
    d["w2"] = _f16(np.transpose(inp["fc2_w"], (0, 2, 1)))
    d["whead"] = _f16(inp["head_w"].T)
    d["qkb"] = _f32(np.asarray(inp["qkv_b"])[:, : 2 * D].reshape(L, 12, 128).transpose(0, 2, 1))
    d["vbrow"] = _f16(np.asarray(inp["qkv_b"])[:, 2 * D :].reshape(L, 1, D))
    d["obrow"] = _f16(np.asarray(inp["out_b"]).reshape(L, 1, D))
    d["f1b"] = _f32(np.asarray(inp["fc1_b"]).reshape(L, KM, 128).transpose(0, 2, 1))
    d["f2row"] = _f16(np.asarray(inp["fc2_b"]).reshape(L, 1, D))
    for nm, key in (("g1", "ln1_g"), ("b1", "ln1_b"), ("g2", "ln2_g"), ("b2", "ln2_b")):
        d[nm] = _f32(np.asarray(inp[key]).reshape(L, KT, 128).transpose(0, 2, 1))
    d["lnfg"] = _f32(np.asarray(inp["lnf_g"]).reshape(KT, 128).T)
    d["lnfb"] = _f32(np.asarray(inp["lnf_b"]).reshape(KT, 128).T)
    d["convb"] = _f32(np.asarray(inp["conv_b"]).reshape(KT, 128).T)
    hb = np.zeros((128, 8), np.float32)
    hb8 = np.asarray(inp["head_b"]).reshape(8, 125)
    for m in range(8):
        hb[:125, m] = hb8[m]
    d["headb"] = hb
    pp = np.zeros((D, T), np.float32)
    pos = np.asarray(inp["pos_emb"][0])
    cls = np.asarray(inp["class_token"][0, 0])
    for i in range(NI):
        pp[:, i * S : (i + 1) * S] = pos.T
        pp[:, i * S] += cls
    d["pos_plus"] = _f32(pp.reshape(KT, 128, T).transpose(1, 0, 2))
    return d


def prep_patches(x):
    n = x.shape[0]
    xr = x.reshape(n, 3, G, P, G, P).transpose(1, 3, 5, 0, 2, 4)
    xr = xr.reshape(3 * P * P, n, G * G)
    out = np.zeros((3 * P * P, n * S), np.float32)
    for i in range(n):
        out[:, i * S + 1 : (i + 1) * S] = xr[:, i, :]
    return _f16(out.reshape(KT, 128, n * S).transpose(1, 0, 2))


def build_program():
    from contextlib import ExitStack

    nc = bacc.Bacc()
    dp = nc.declare_dram_parameter
    xp = dp("xp", [128, KT, T], F16, isOutput=False)
    tp = dp("tp", [128, KT, T], F16, isOutput=False)
    wconv = dp("wconv", [D, D], F16, isOutput=False)
    wqk = dp("wqk", [L, D, 2 * D], F16, isOutput=False)
    wv = dp("wv", [L, D, D], F16, isOutput=False)
    wo = dp("wo", [L, D, D], F16, isOutput=False)
    w1 = dp("w1", [L, D, MLPD], F16, isOutput=False)
    w2 = dp("w2", [L, MLPD, D], F16, isOutput=False)
    whead = dp("whead", [D, 1000], F16, isOutput=False)
    qkb = dp("qkb", [L, 128, 12], F32, isOutput=False)
    vbrow = dp("vbrow", [L, 1, D], F16, isOutput=False)
    obrow = dp("obrow", [L, 1, D], F16, isOutput=False)
    f1b = dp("f1b", [L, 128, KM], F32, isOutput=False)
    f2row = dp("f2row", [L, 1, D], F16, isOutput=False)
    lng = {1: dp("g1", [L, 128, KT], F32, isOutput=False), 2: dp("g2", [L, 128, KT], F32, isOutput=False)}
    lnb = {1: dp("b1", [L, 128, KT], F32, isOutput=False), 2: dp("b2", [L, 128, KT], F32, isOutput=False)}
    lnfg = dp("lnfg", [128, KT], F32, isOutput=False)
    lnfb = dp("lnfb", [128, KT], F32, isOutput=False)
    convb = dp("convb", [128, KT], F32, isOutput=False)
    headb = dp("headb", [128, 8], F32, isOutput=False)
    pos_plus = dp("pos_plus", [128, KT, T], F32, isOutput=False)
    out_d = dp("out", [NI, 1000], F32, isOutput=True)
    jvp_d = dp("out_jvp", [NI, 1000], F32, isOutput=True)
    DBG = os.environ.get("VIT_DEBUG", "0") == "1"
    if DBG:
        dbg = {nm: dp(f"d_{nm}", [128, KT, T], F32 if nm.startswith("z") or nm.startswith("tz") else F16, isOutput=True)
               for nm in ("z0", "tz0", "y", "ty", "o", "to", "z1", "tz1", "z2", "tz2")}

    with tile.TileContext(nc) as tc, ExitStack() as ctx:
        res = ctx.enter_context(tc.tile_pool(name="res", bufs=1))
        act = ctx.enter_context(tc.tile_pool(name="act", bufs=1))
        sh = ctx.enter_context(tc.tile_pool(name="sh", bufs=1))
        wpool = ctx.enter_context(tc.tile_pool(name="wp", bufs=2))
        small = ctx.enter_context(tc.tile_pool(name="sm", bufs=2))
        rows = ctx.enter_context(tc.tile_pool(name="rows", bufs=1))
        cst = ctx.enter_context(tc.tile_pool(name="cst", bufs=1))
        pm = ctx.enter_context(tc.tile_pool(name="pm", bufs=4, space="PSUM"))
        pbc = ctx.enter_context(tc.tile_pool(name="pbc", bufs=2, space="PSUM"))
        psc = ctx.enter_context(tc.tile_pool(name="psc", bufs=2, space="PSUM"))

        P_ = {"pm": lambda s_: pm.tile(s_, F32, tag="pm", name="pm"),
              "bc": lambda s_: pbc.tile(s_, F32, tag="bc", name="bc"),
              "sc": lambda s_: psc.tile(s_, F32, tag="sc", name="sc")}

        ones_r = cst.tile([1, TC0], F16, tag="ones_r", name="ones_r")
        nc.vector.memset(ones_r, 1.0)
        ones_c = cst.tile([128, 1], F16, tag="ones_c", name="ones_c")
        nc.vector.memset(ones_c, 1.0)
        ones_m = cst.tile([1, 128], F16, tag="ones_m", name="ones_m")
        nc.vector.memset(ones_m, 1.0)
        epst = cst.tile([1, 1], F32, tag="eps", name="eps")
        nc.vector.memset(epst, EPS)
        convb_t = cst.tile([128, KT], F32, tag="convb", name="convb")
        nc.sync.dma_start(out=convb_t, in_=convb[:, :])
        lnfg_t = cst.tile([128, KT], F32, tag="lnfg", name="lnfg")
        nc.sync.dma_start(out=lnfg_t, in_=lnfg[:, :])
        lnfb_t = cst.tile([128, KT], F32, tag="lnfb", name="lnfb")
        nc.sync.dma_start(out=lnfb_t, in_=lnfb[:, :])
        headb_t = cst.tile([128, 8], F32, tag="headb", name="headb")
        nc.sync.dma_start(out=headb_t, in_=headb[:, :])

        z = [res.tile([128, T], F32, tag=f"z{k}", name=f"z{k}") for k in range(KT)]
        tz = [res.tile([128, T], F32, tag=f"tz{k}", name=f"tz{k}") for k in range(KT)]
        z16 = [act.tile([128, T], F16, tag=f"z16_{k}", name=f"z16_{k}") for k in range(KT)]
        tz16 = [act.tile([128, T], F16, tag=f"tz16_{k}", name=f"tz16_{k}") for k in range(KT)]
        y16 = [act.tile([128, T], F16, tag=f"y16_{k}", name=f"y16_{k}") for k in range(KT)]
        ty16 = [act.tile([128, T], F16, tag=f"ty16_{k}", name=f"ty16_{k}") for k in range(KT)]

        def T1(shape, dt=F16):
            return sh.tile(shape, dt, tag="T1", name="T1")

        def T2(shape, dt=F16):
            return sh.tile(shape, dt, tag="T2", name="T2")

        def T3(shape, dt=F16):
            return sh.tile(shape, dt, tag="T3", name="T3")

        def T4(shape, dt=F16):
            return sh.tile(shape, dt, tag="T4", name="T4", bufs=2)

        # ---------------- patch embedding ----------------
        xp16 = T1([128, KT, T])
        tp16 = T2([128, KT, T])
        nc.sync.dma_start(out=xp16, in_=xp[:, :, :])
        nc.sync.dma_start(out=tp16, in_=tp[:, :, :])
        for m in range(KT):
            wt = wpool.tile([128, KT, 128], F16, tag="w", name="w")
            nc.sync.dma_start(out=wt, in_=wconv.rearrange("(kt p) e -> p kt e", p=128)[:, :, m * 128 : (m + 1) * 128])
            for i in range(NI):
                ps_p = P_["pm"]([128, S])
                ps_t = P_["pm"]([128, S])
                for kt in range(KT):
                    nc.tensor.matmul(ps_p[:, :], wt[:, kt, :], xp16[:, kt, i * S : (i + 1) * S],
                                     start=(kt == 0), stop=(kt == KT - 1))
                for kt in range(KT):
                    nc.tensor.matmul(ps_t[:, :], wt[:, kt, :], tp16[:, kt, i * S : (i + 1) * S],
                                     start=(kt == 0), stop=(kt == KT - 1))
                nc.vector.tensor_scalar(out=z[m][:, i * S + 1 : (i + 1) * S], in0=ps_p[:, 1:S],
                                        scalar1=convb_t[:, m : m + 1], scalar2=None, op0=OP.add)
                nc.vector.tensor_copy(out=z[m][:, i * S : i * S + 1], in_=ps_p[:, 0:1])
                nc.vector.tensor_copy(out=tz[m][:, i * S : (i + 1) * S], in_=ps_t[:, :])
        for half in range(NCH):
            ppt = T3([128, KT, TC0], F32)
            nc.sync.dma_start(out=ppt, in_=pos_plus[:, :, half * TC0 : (half + 1) * TC0])
            for m in range(KT):
                nc.vector.tensor_add(out=z[m][:, half * TC0 : (half + 1) * TC0],
                                     in0=z[m][:, half * TC0 : (half + 1) * TC0], in1=ppt[:, m, :])

        # ---------------- layernorm ----------------
        def layernorm(which, l, yn16):
            gt = small.tile([128, KT], F32, tag="gt", name="gt")
            bt = small.tile([128, KT], F32, tag="bt", name="bt")
            nc.sync.dma_start(out=gt, in_=lng[which][l, :, :])
            nc.sync.dma_start(out=bt, in_=lnb[which][l, :, :])
            for k in range(KT):
                nc.gpsimd.tensor_copy(out=z16[k], in_=z[k])
                nc.gpsimd.tensor_copy(out=tz16[k], in_=tz[k])
            prs = [P_[p]([1, TC0]) for p in ("pm", "pm", "pm", "pm", "bc", "bc", "sc", "sc")]
            for si in range(4):
                for nchi in range(NCH):
                    sl = slice(nchi * TC0, (nchi + 1) * TC0)
                    for k in range(KT):
                        if si == 0:
                            opnd = z16[k][:, sl]
                        elif si == 2:
                            opnd = tz16[k][:, sl]
                        else:
                            opnd = small.tile([128, TC0], F16, tag="sq", bufs=2, name="sq")
                            nc.vector.tensor_mul(out=opnd, in0=z16[k][:, sl],
                                                 in1=(z16[k][:, sl] if si == 1 else tz16[k][:, sl]))
                        nc.tensor.matmul(prs[2 * si + nchi][0:1, :], ones_c, opnd,
                                         start=(k == 0), stop=(k == KT - 1))

            def row(tag, dt=F32):
                return rows.tile([1, T], dt, tag=tag, name=tag)

            mu, q2, tmu, cov = row("mu"), row("q2"), row("tmu"), row("cov")
            rstd = row("rstd")
            for nchi in range(NCH):
                sl = slice(nchi * TC0, (nchi + 1) * TC0)
                nc.scalar.mul(out=mu[0:1, sl], in_=prs[0 + nchi][0:1, :], mul=1.0 / D)
                nc.scalar.mul(out=q2[0:1, sl], in_=prs[2 + nchi][0:1, :], mul=1.0 / D)
                nc.scalar.mul(out=tmu[0:1, sl], in_=prs[4 + nchi][0:1, :], mul=1.0 / D)
                nc.scalar.mul(out=cov[0:1, sl], in_=prs[6 + nchi][0:1, :], mul=1.0 / D)
            nc.vector.tensor_mul(out=rstd, in0=mu, in1=mu)
            nc.vector.tensor_sub(out=q2, in0=q2, in1=rstd)         # var
            nc.scalar.activation(out=q2, in_=q2, func=AF.Sqrt, bias=epst[0:1, 0:1])
            nc.vector.reciprocal(out=rstd, in_=q2)
            nc.vector.tensor_mul(out=q2, in0=mu, in1=tmu)
            nc.vector.tensor_sub(out=cov, in0=cov, in1=q2)         # cov
            rstd16 = row("rstd16", F16)
            mur16 = row("mur16", F16)
            tmur16 = row("tmur16", F16)
            g16 = row("g16", F16)
            nc.vector.tensor_copy(out=rstd16, in_=rstd)
            nc.vector.tensor_mul(out=mur16, in0=mu, in1=rstd)
            nc.vector.tensor_mul(out=tmur16, in0=tmu, in1=rstd)
            nc.vector.tensor_mul(out=q2, in0=rstd, in1=rstd)
            nc.vector.tensor_mul(out=g16, in0=cov, in1=q2)
            for nchi in range(NCH):
                c0 = nchi * TC0
                sl = slice(c0, c0 + TC0)
                pb_r, pb_m = P_["bc"]([128, TC0]), P_["sc"]([128, TC0])
                nc.tensor.matmul(pb_r[:, :], ones_m, rstd16[0:1, sl], start=True, stop=True)
                nc.tensor.matmul(pb_m[:, :], ones_m, mur16[0:1, sl], start=True, stop=True)
                for k in range(KT):
                    ynk = yn16[:, k, sl]
                    nc.vector.tensor_tensor(out=ynk, in0=z16[k][:, sl], in1=pb_r[:, :], op=OP.mult)
                    nc.vector.tensor_tensor(out=ynk, in0=ynk, in1=pb_m[:, :], op=OP.subtract)
                    nc.vector.tensor_scalar(out=y16[k][:, sl], in0=ynk, scalar1=gt[:, k : k + 1],
                                            scalar2=bt[:, k : k + 1], op0=OP.mult, op1=OP.add)
                pb_t, pb_g = P_["bc"]([128, TC0]), P_["sc"]([128, TC0])
                nc.tensor.matmul(pb_t[:, :], ones_m, tmur16[0:1, sl], start=True, stop=True)
                nc.tensor.matmul(pb_g[:, :], ones_m, g16[0:1, sl], start=True, stop=True)
                for k in range(KT):
                    tyk = ty16[k][:, sl]
                    tg = small.tile([128, TC0], F16, tag="tg", name="tg")
                    nc.vector.tensor_tensor(out=tyk, in0=tz16[k][:, sl], in1=pb_r[:, :], op=OP.mult)
                    nc.vector.tensor_tensor(out=tyk, in0=tyk, in1=pb_t[:, :], op=OP.subtract)
                    nc.vector.tensor_tensor(out=tg, in0=yn16[:, k, sl], in1=pb_g[:, :], op=OP.mult)
                    nc.vector.tensor_sub(out=tyk, in0=tyk, in1=tg)
                    nc.vector.tensor_scalar(out=tyk, in0=tyk, scalar1=gt[:, k : k + 1],
                                            scalar2=None, op0=OP.mult)

        def dump(nm, tiles):
            if DBG:
                for k in range(KT):
                    nc.sync.dma_start(out=dbg[nm][:, k, :], in_=tiles[k])

        dump("z0", z)
        dump("tz0", tz)

        # ---------------- layers ----------------
        for l in range(L_RUN):
            yn = T1([128, KT, T])
            layernorm(1, l, yn)
            if l == 0:
                dump("y", y16)
                dump("ty", ty16)
            o16 = z16
            to16 = tz16
            qkbt = small.tile([128, 12], F32, tag="qkb", name="qkb")
            nc.sync.dma_start(out=qkbt, in_=qkb[l, :, :])
            vbr = small.tile([1, D], F16, tag="vbr", name="vbr")
            nc.sync.dma_start(out=vbr, in_=vbrow[l, :, :])
            for i in range(NI):
                i0 = i * S
                qki = T3([128, 24, S])   # m 0..11 primal, 12..23 tangent
                for m in range(12):
                    wt = wpool.tile([128, KT, 128], F16, tag="w", name="w")
                    nc.sync.dma_start(out=wt, in_=wqk[l].rearrange("(kt p) e -> p kt e", p=128)[:, :, m * 128 : (m + 1) * 128])
                    ps_p = P_["pm"]([128, S])
                    ps_t = P_["pm"]([128, S])
                    for kt in range(KT):
                        nc.tensor.matmul(ps_p[:, :], wt[:, kt, :], y16[kt][:, i0 : i0 + S],
                                         start=(kt == 0), stop=(kt == KT - 1))
                    for kt in range(KT):
                        nc.tensor.matmul(ps_t[:, :], wt[:, kt, :], ty16[kt][:, i0 : i0 + S],
                                         start=(kt == 0), stop=(kt == KT - 1))
                    nc.vector.tensor_scalar(out=qki[:, m, :], in0=ps_p[:, :],
                                            scalar1=qkbt[:, m : m + 1], scalar2=None, op0=OP.add)
                    nc.vector.tensor_copy(out=qki[:, 12 + m, :], in_=ps_t[:, :])
                vti = T4([128, 4, D])    # [primal c0, primal c1, tang c0, tang c1]
                for nchi in range(2):
                    wvt = wpool.tile([128, KT, 384], F16, tag="w", name="w")
                    nc.sync.dma_start(out=wvt, in_=wv[l].rearrange("(kt p) e -> p kt e", p=128)[:, :, nchi * 384 : (nchi + 1) * 384])
                    for jvp in range(2):
                        src = y16 if jvp == 0 else ty16
                        for mc in range(2):
                            mr = 128 if mc == 0 else S - 128
                            t0 = i0 + mc * 128
                            pv = P_["pm"]([128, TC0])
                            for kt in range(KT):
                                nc.tensor.matmul(pv[0:mr, 0:384], src[kt][:, t0 : t0 + mr], wvt[:, kt, :],
                                                 start=(kt == 0), stop=(kt == KT - 1 and jvp == 1))
                            if jvp == 0:
                                nc.tensor.matmul(pv[0:mr, 0:384], ones_m[0:1, 0:mr],
                                                 vbr[0:1, nchi * 384 : (nchi + 1) * 384], start=False, stop=True)
                            nc.vector.tensor_copy(out=vti[0:mr, 2 * jvp + mc, nchi * 384 : (nchi + 1) * 384],
                                                  in_=pv[0:mr, 0:384])

                # scores / softmax / JVP / AV for this image
                ets = T1([128, 4, 12 * S])   # e0, e1, ts0, ts1
                att = T2([128, 2, 24 * S])   # [A(12S) | TA(12S)] per sk-chunk
                rrec = rows.tile([1, 12 * S], F16, tag="rrec", name="rrec")
                rr = rows.tile([1, 12 * S], F16, tag="rr", name="rr")
                for h in range(12):
                    qa = qki[(h % 2) * 64 : (h % 2) * 64 + 64, h // 2, :]
                    tqa = qki[(h % 2) * 64 : (h % 2) * 64 + 64, 12 + h // 2, :]
                    for c in range(2):
                        skn = 128 if c == 0 else S - 128
                        ka = qki[(h % 2) * 64 : (h % 2) * 64 + 64, 6 + h // 2, c * 128 : c * 128 + skn]
                        tka = qki[(h % 2) * 64 : (h % 2) * 64 + 64, 18 + h // 2, c * 128 : c * 128 + skn]
                        pss = P_["sc"]([128, 2 * S])
                        rhs_qtq = bass.AP(tensor=qa.tensor, offset=qa.offset,
                                          ap=[list(qa.ap[0]), [12 * S, 2], [1, S]])
                        nc.tensor.matmul(pss[0:skn, 0 : 2 * S], ka, rhs_qtq, start=True, stop=False)
                        nc.tensor.matmul(pss[0:skn, S : 2 * S], tka, qa, start=False, stop=True)
                        nc.scalar.activation(out=ets[0:skn, c, h * S : (h + 1) * S], in_=pss[0:skn, 0:S],
                                             func=AF.Exp, scale=0.125)
                        nc.vector.tensor_single_scalar(out=ets[0:skn, 2 + c, h * S : (h + 1) * S],
                                                       in_=pss[0:skn, S : 2 * S], scalar=0.125, op=OP.mult)
                    pd = P_["bc"]([1, 2 * S])
                    fhs = []
                    for c in range(2):
                        skn = 128 if c == 0 else S - 128
                        fh = small.tile([128, S], F16, tag="fh", name="fh")
                        nc.vector.tensor_mul(out=fh[0:skn, :], in0=ets[0:skn, c, h * S : (h + 1) * S],
                                             in1=ets[0:skn, 2 + c, h * S : (h + 1) * S])
                        fhs.append(fh)
                    for c in range(2):
                        skn = 128 if c == 0 else S - 128
                        nc.tensor.matmul(pd[0:1, 0:S], ones_c[0:skn, :], ets[0:skn, c, h * S : (h + 1) * S],
                                         start=(c == 0), stop=(c == 1))
                    for c in range(2):
                        skn = 128 if c == 0 else S - 128
                        nc.tensor.matmul(pd[0:1, S : 2 * S], ones_c[0:skn, :], fhs[c][0:skn, :],
                                         start=(c == 0), stop=(c == 1))
                    with nc.allow_low_precision(reason="softmax denom recip to fp16 is intentional"):
                        nc.vector.reciprocal(out=rrec[0:1, h * S : (h + 1) * S], in_=pd[0:1, 0:S])
                    nc.vector.tensor_tensor(out=rr[0:1, h * S : (h + 1) * S], in0=pd[0:1, S : 2 * S],
                                            in1=rrec[0:1, h * S : (h + 1) * S], op=OP.mult)
                for cc in range(6):
                    c0 = cc * TC0
                    pb1, pb2 = P_["bc"]([128, TC0]), P_["sc"]([128, TC0])
                    nc.tensor.matmul(pb1[:, :], ones_m, rrec[0:1, c0 : c0 + TC0], start=True, stop=True)
                    nc.tensor.matmul(pb2[:, :], ones_m, rr[0:1, c0 : c0 + TC0], start=True, stop=True)
                    for c in range(2):
                        skn = 128 if c == 0 else S - 128
                        nc.vector.tensor_tensor(out=att[0:skn, c, c0 : c0 + TC0], in0=ets[0:skn, c, c0 : c0 + TC0],
                                                in1=pb1[0:skn, :], op=OP.mult)
                        nc.vector.tensor_tensor(out=ets[0:skn, 2 + c, c0 : c0 + TC0],
                                                in0=ets[0:skn, 2 + c, c0 : c0 + TC0],
                                                in1=pb2[0:skn, :], op=OP.subtract)
                        nc.vector.tensor_mul(out=att[0:skn, c, 12 * S + c0 : 12 * S + c0 + TC0],
                                             in0=att[0:skn, c, c0 : c0 + TC0],
                                             in1=ets[0:skn, 2 + c, c0 : c0 + TC0])
                for h in range(12):
                    po = P_["pm"]([64, 2 * S])
                    for c in range(2):
                        skn = 128 if c == 0 else S - 128
                        base = att[0:skn, c, h * S : h * S + S]
                        rhs2 = bass.AP(tensor=base.tensor, offset=base.offset,
                                       ap=[list(base.ap[0]), [12 * S, 2], [1, S]])
                        nc.tensor.matmul(po[0:64, 0 : 2 * S], vti[0:skn, c, h * 64 : (h + 1) * 64],
                                         rhs2, start=(c == 0), stop=False)
                    for c in range(2):
                        skn = 128 if c == 0 else S - 128
                        nc.tensor.matmul(po[0:64, S : 2 * S], vti[0:skn, 2 + c, h * 64 : (h + 1) * 64],
                                         att[0:skn, c, h * S : (h + 1) * S], start=False, stop=(c == 1))
                    nc.vector.tensor_copy(out=o16[h // 2][(h % 2) * 64 : (h % 2) * 64 + 64, i0 : i0 + S],
                                          in_=po[0:64, 0:S])
                    nc.vector.tensor_copy(out=to16[h // 2][(h % 2) * 64 : (h % 2) * 64 + 64, i0 : i0 + S],
                                          in_=po[0:64, S : 2 * S])

            # ---- output projection + residual ----
            obr = small.tile([1, D], F16, tag="obr", name="obr")
            nc.sync.dma_start(out=obr, in_=obrow[l, :, :])
            for m in range(KT):
                wt = wpool.tile([128, KT, 128], F16, tag="w", name="w")
                nc.sync.dma_start(out=wt, in_=wo[l].rearrange("(kt p) e -> p kt e", p=128)[:, :, m * 128 : (m + 1) * 128])
                for jvp in range(2):
                    src, tgt = (o16, z) if jvp == 0 else (to16, tz)
                    for nchi in range(NCH):
                        sl = slice(nchi * TC0, (nchi + 1) * TC0)
                        pp_ = P_["pm"]([128, TC0])
                        for kt in range(KT):
                            nc.tensor.matmul(pp_[:, :], wt[:, kt, :], src[kt][:, sl],
                                             start=(kt == 0), stop=(kt == KT - 1 and jvp == 1))
                        if jvp == 0:
                            nc.tensor.matmul(pp_[:, :], obr[0:1, m * 128 : (m + 1) * 128], ones_r,
                                             start=False, stop=True)
                        nc.vector.tensor_tensor(out=tgt[m][:, sl], in0=tgt[m][:, sl], in1=pp_[:, :], op=OP.add)

            if l == 0:
                dump("z1", z)
                dump("tz1", tz)
            # ---- LN2 + MLP ----
            yn2 = T1([128, KT, T])
            layernorm(2, l, yn2)
            f1bt = small.tile([128, KM], F32, tag="f1b", name="f1b")
            nc.sync.dma_start(out=f1bt, in_=f1b[l, :, :])
            f2r = small.tile([1, D], F16, tag="f2r", name="f2r")
            nc.sync.dma_start(out=f2r, in_=f2row[l, :, :])
            for half in range(NCH):
                c0 = half * TC0
                sl = slice(c0, c0 + TC0)
                h16 = T1([128, KM, TC0])
                th16 = T2([128, KM, TC0])
                for m in range(KM):
                    wt = wpool.tile([128, KT, 128], F16, tag="w", name="w")
                    nc.sync.dma_start(out=wt, in_=w1[l].rearrange("(kt p) e -> p kt e", p=128)[:, :, m * 128 : (m + 1) * 128])
                    pp_ = P_["pm"]([128, TC0])
                    pt_ = P_["pm"]([128, TC0])
                    for kt in range(KT):
                        nc.tensor.matmul(pp_[:, :], wt[:, kt, :], y16[kt][:, sl],
                                         start=(kt == 0), stop=(kt == KT - 1))
                    for kt in range(KT):
                        nc.tensor.matmul(pt_[:, :], wt[:, kt, :], ty16[kt][:, sl],
                                         start=(kt == 0), stop=(kt == KT - 1))
                    nc.scalar.activation(out=h16[:, m, :], in_=pp_[:, :], func=AF.Gelu, bias=f1bt[:, m : m + 1])
                    dg = small.tile([128, TC0], F16, tag="dg", name="dg")
                    nc.scalar.activation(out=dg, in_=pp_[:, :], func=AF.Derivative_Gelu, bias=f1bt[:, m : m + 1])
                    nc.vector.tensor_tensor(out=th16[:, m, :], in0=dg, in1=pt_[:, :], op=OP.mult)
                for m in range(KT):
                    wt = wpool.tile([128, KM, 128], F16, tag="w", name="w")
                    nc.sync.dma_start(out=wt, in_=w2[l].rearrange("(kt p) e -> p kt e", p=128)[:, :, m * 128 : (m + 1) * 128])
                    pp_ = P_["pm"]([128, TC0])
                    pt_ = P_["pm"]([128, TC0])
                    for kt in range(KM):
                        nc.tensor.matmul(pp_[:, :], wt[:, kt, :], h16[:, kt, :], start=(kt == 0), stop=False)
                    nc.tensor.matmul(pp_[:, :], f2r[0:1, m * 128 : (m + 1) * 128], ones_r, start=False, stop=True)
                    for kt in range(KM):
                        nc.tensor.matmul(pt_[:, :], wt[:, kt, :], th16[:, kt, :],
                                         start=(kt == 0), stop=(kt == KM - 1))
                    nc.vector.tensor_tensor(out=z[m][:, sl], in0=z[m][:, sl], in1=pp_[:, :], op=OP.add)
                    nc.vector.tensor_tensor(out=tz[m][:, sl], in0=tz[m][:, sl], in1=pt_[:, :], op=OP.add)

        if DBG and L_RUN > 0:
            dump("z2", z)
            dump("tz2", tz)
        # ---------------- final LN + head ----------------
        zc = small.tile([128, KT, 8], F16, tag="zc", name="zc")
        sq8 = small.tile([128, KT, 8], F16, tag="sq8", name="sq8")
        for k in range(KT):
            ap_z = bass.AP(tensor=z[k].tensor, offset=z[k].offset, ap=[list(z[k][:, 0:1].ap[0]), [S, NI]])
            ap_t = bass.AP(tensor=tz[k].tensor, offset=tz[k].offset, ap=[list(tz[k][:, 0:1].ap[0]), [S, NI]])
            nc.vector.tensor_copy(out=zc[:, k, 0:4], in_=ap_z)
            nc.vector.tensor_copy(out=zc[:, k, 4:8], in_=ap_t)
            nc.vector.tensor_mul(out=sq8[:, k, 0:4], in0=zc[:, k, 0:4], in1=zc[:, k, 0:4])
            nc.vector.tensor_mul(out=sq8[:, k, 4:8], in0=zc[:, k, 0:4], in1=zc[:, k, 4:8])
        p8a = P_["bc"]([1, 8])
        p8b = P_["sc"]([1, 8])
        for k in range(KT):
            nc.tensor.matmul(p8a[0:1, :], ones_c, zc[:, k, :], start=(k == 0), stop=(k == KT - 1))
        for k in range(KT):
            nc.tensor.matmul(p8b[0:1, :], ones_c, sq8[:, k, :], start=(k == 0), stop=(k == KT - 1))
        r8 = rows.tile([1, 24], F32, tag="r8", name="r8")
        nc.scalar.mul(out=r8[0:1, 0:8], in_=p8a[0:1, :], mul=1.0 / D)
        nc.scalar.mul(out=r8[0:1, 8:16], in_=p8b[0:1, :], mul=1.0 / D)
        mu8, tmu8 = r8[0:1, 0:4], r8[0:1, 4:8]
        q28, cv8 = r8[0:1, 8:12], r8[0:1, 12:16]
        var8, rstd8 = r8[0:1, 16:20], r8[0:1, 20:24]
        nc.vector.tensor_mul(out=var8, in0=mu8, in1=mu8)
        nc.vector.tensor_sub(out=var8, in0=q28, in1=var8)
        nc.scalar.activation(out=var8, in_=var8, func=AF.Sqrt, bias=epst[0:1, 0:1])
        nc.vector.reciprocal(out=rstd8, in_=var8)
        r8b = rows.tile([1, 24], F16, tag="r8b", name="r8b")
        nc.vector.tensor_copy(out=r8b[0:1, 0:4], in_=rstd8)
        nc.vector.tensor_copy(out=r8b[0:1, 4:8], in_=rstd8)
        nc.vector.tensor_mul(out=r8b[0:1, 8:12], in0=mu8, in1=rstd8)
        nc.vector.tensor_mul(out=q28, in0=mu8, in1=tmu8)
        nc.vector.tensor_sub(out=cv8, in0=cv8, in1=q28)
        nc.vector.tensor_mul(out=r8b[0:1, 12:16], in0=tmu8, in1=rstd8)
        nc.vector.tensor_mul(out=q28, in0=rstd8, in1=rstd8)
        nc.vector.tensor_mul(out=cv8, in0=cv8, in1=q28)
        nc.vector.tensor_copy(out=r8b[0:1, 16:20], in_=cv8)
        pbA = P_["bc"]([128, 8])
        pbB = P_["sc"]([128, 8])
        pbG = P_["bc"]([128, 4])
        nc.tensor.matmul(pbA[:, :], ones_m, r8b[0:1, 0:8], start=True, stop=True)
        nc.tensor.matmul(pbB[:, :], ones_m, r8b[0:1, 8:16], start=True, stop=True)
        nc.tensor.matmul(pbG[:, :], ones_m, r8b[0:1, 16:20], start=True, stop=True)
        y0 = small.tile([128, KT, 8], F16, tag="y0", name="y0")
        for k in range(KT):
            w0 = small.tile([128, 8], F16, tag="w0", name="w0")
            tg8 = small.tile([128, 4], F16, tag="tg8", name="tg8")
            nc.vector.tensor_tensor(out=w0, in0=zc[:, k, :], in1=pbA[:, :], op=OP.mult)
            nc.vector.tensor_tensor(out=w0, in0=w0, in1=pbB[:, :], op=OP.subtract)
            nc.vector.tensor_tensor(out=tg8, in0=w0[:, 0:4], in1=pbG[:, :], op=OP.mult)
            nc.vector.tensor_sub(out=w0[:, 4:8], in0=w0[:, 4:8], in1=tg8)
            nc.vector.tensor_scalar(out=y0[:, k, 0:4], in0=w0[:, 0:4], scalar1=lnfg_t[:, k : k + 1],
                                    scalar2=lnfb_t[:, k : k + 1], op0=OP.mult, op1=OP.add)
            nc.vector.tensor_scalar(out=y0[:, k, 4:8], in0=w0[:, 4:8], scalar1=lnfg_t[:, k : k + 1],
                                    scalar2=None, op0=OP.mult)
        for m in range(8):
            wt = wpool.tile([128, KT, 125], F16, tag="wh", name="wh")
            nc.sync.dma_start(out=wt, in_=whead.rearrange("(kt p) e -> p kt e", p=128)[:, :, m * 125 : (m + 1) * 125])
            ph = P_["pm"]([125, 8])
            for kt in range(KT):
                nc.tensor.matmul(ph[0:125, :], wt[:, kt, :], y0[:, kt, :], start=(kt == 0), stop=(kt == KT - 1))
            lg = small.tile([128, 8], F32, tag="lg", name="lg")
            nc.vector.tensor_scalar(out=lg[0:125, 0:4], in0=ph[0:125, 0:4],
                                    scalar1=headb_t[:125, m : m + 1], scalar2=None, op0=OP.add)
            nc.vector.tensor_copy(out=lg[0:125, 4:8], in_=ph[0:125, 4:8])
            o_ap = bass.AP(tensor=out_d, offset=m * 125, ap=[[1, 125], [1000, NI]])
            j_ap = bass.AP(tensor=jvp_d, offset=m * 125, ap=[[1, 125], [1000, NI]])
            nc.sync.dma_start(out=o_ap, in_=lg[0:125, 0:4])
            nc.sync.dma_start(out=j_ap, in_=lg[0:125, 4:8])

    nc.compile()
    return nc


_PROG = None


def kernel(**inputs):
    global _PROG
    from concourse.bass_utils import run_bass_kernel_spmd

    inputs = {k: np.asarray(v) for k, v in inputs.items()}
    host = prep_host(inputs)
    if _PROG is None:
        _PROG = build_program()
    in_maps = []
    for c in range(NCORES):
        m = dict(host)
        m["xp"] = prep_patches(np.asarray(inputs["x"][c * NI : (c + 1) * NI], np.float32))
        m["tp"] = prep_patches(np.asarray(inputs["tangent"][c * NI : (c + 1) * NI], np.float32))
        in_maps.append(m)
    res_ = run_bass_kernel_spmd(_PROG, in_maps, list(range(NCORES)))
    out = np.concatenate([res_.results[c]["out"] for c in range(NCORES)], axis=0)
    jvp = np.concatenate([res_.results[c]["out_jvp"] for c in range(NCORES)], axis=0)
    return out.astype(np.float32), jvp.astype(np.float32)



# revision 12
# speedup vs baseline: 1.0727x; 1.0727x over previous
"""ViT-Base forward + JVP on 8 trn2 NeuronCores, data-parallel over batch.

v2: fp16 residual (z16/tz16 only), LN apply via scalar_tensor_tensor (DVE 4x
mode, SBUF fp16 broadcast tiles), weights loaded once per layer with 512B-
contiguous DMA layouts, 0.125 folded into wq, conv bias folded into pos_emb,
elementwise work spread across DVE / Act / Pool engines.
"""

import os
import sys

sys.path.insert(0, "/opt/trn_rl_repo")

import numpy as np

from concourse import bacc, bass, mybir, tile

F32 = mybir.dt.float32
F16 = mybir.dt.float16
AF = mybir.ActivationFunctionType
OP = mybir.AluOpType

IMG, P, L, HEADS, D, MLPD, NCLS, B = 224, 16, 12, 12, 768, 3072, 1000, 32
G = IMG // P
S = G * G + 1            # 197
DH = D // HEADS          # 64
EPS = 1e-6
NCORES = 8
NI = B // NCORES         # 4
T = NI * S               # 788
KT = D // 128            # 6
KM = MLPD // 128         # 24
NCH = 2
TC0 = T // NCH           # 394
L_RUN = int(os.environ.get("VIT_LAYERS", "12"))


def _f16(a):
    return np.ascontiguousarray(a, dtype=np.float16)


def _f32(a):
    return np.ascontiguousarray(a, dtype=np.float32)


def _wlayout(w):
    """[L, E, Din] -> [L, 128, Din//128, E] so DMA slices are contiguous."""
    Lx, E, Din = w.shape
    return _f16(np.transpose(w, (0, 2, 1)).reshape(Lx, Din // 128, 128, E).transpose(0, 2, 1, 3))


def prep_host(inp):
    d = {}
    wc = np.asarray(inp["conv_w"]).reshape(D, 3 * P * P).T  # [dIn=768, e=768]
    d["wconv"] = _f16(wc.reshape(KT, 128, D).transpose(1, 0, 2))
    qkv = np.asarray(inp["qkv_w"])           # [L, 3D, D]
    wqk = np.array(qkv[:, : 2 * D, :])       # [L, 2D, D]
    wqk[:, :D, :] *= 0.125                   # fold score scale into q
    d["wqk"] = _wlayout(wqk)
    d["wv"] = _wlayout(qkv[:, 2 * D :, :])
    d["wo"] = _wlayout(np.asarray(inp["out_w"]))
    d["w1"] = _wlayout(np.asarray(inp["fc1_w"]))
    d["w2"] = _wlayout(np.asarray(inp["fc2_w"]))
    hw = np.asarray(inp["head_w"]).T          # [D, 1000]
    d["whead"] = _f16(hw.reshape(KT, 128, NCLS).transpose(1, 0, 2))
    qkb = np.array(np.asarray(inp["qkv_b"])[:, : 2 * D])
    qkb[:, :D] *= 0.125
    d["qkb"] = _f32(qkb.reshape(L, 12, 128).transpose(0, 2, 1))
    d["vbrow"] = _f16(np.asarray(inp["qkv_b"])[:, 2 * D :].reshape(L, 1, D))
    d["obrow"] = _f16(np.asarray(inp["out_b"]).reshape(L, 1, D))
    d["f1b"] = _f32(np.asarray(inp["fc1_b"]).reshape(L, KM, 128).transpose(0, 2, 1))
    d["f2row"] = _f16(np.asarray(inp["fc2_b"]).reshape(L, 1, D))
    for nm, key in (("g1", "ln1_g"), ("b1", "ln1_b"), ("g2", "ln2_g"), ("b2", "ln2_b")):
        d[nm] = _f32(np.asarray(inp[key]).reshape(L, KT, 128).transpose(0, 2, 1))
    d["lnfg"] = _f32(np.asarray(inp["lnf_g"]).reshape(KT, 128).T)
    d["lnfb"] = _f32(np.asarray(inp["lnf_b"]).reshape(KT, 128).T)
    hb = np.zeros((128, 8), np.float32)
    hb8 = np.asarray(inp["head_b"]).reshape(8, 125)
    for m in range(8):
        hb[:125, m] = hb8[m]
    d["headb"] = hb
    # pos emb + class token + conv bias (conv bias on patch tokens only)
    pp = np.zeros((D, T), np.float32)
    pos = np.asarray(inp["pos_emb"][0])
    cls = np.asarray(inp["class_token"][0, 0])
    cb = np.asarray(inp["conv_b"])
    for i in range(NI):
        pp[:, i * S : (i + 1) * S] = pos.T
        pp[:, i * S] += cls
        pp[:, i * S + 1 : (i + 1) * S] += cb[:, None]
    d["pos_plus"] = _f16(pp.reshape(KT, 128, T).transpose(1, 0, 2))
    return d


def prep_patches(x):
    n = x.shape[0]
    xr = x.reshape(n, 3, G, P, G, P).transpose(1, 3, 5, 0, 2, 4)
    xr = xr.reshape(3 * P * P, n, G * G)
    out = np.zeros((3 * P * P, n * S), np.float32)
    for i in range(n):
        out[:, i * S + 1 : (i + 1) * S] = xr[:, i, :]
    return _f16(out.reshape(KT, 128, n * S).transpose(1, 0, 2))


def build_program():
    from contextlib import ExitStack

    nc = bacc.Bacc()
    dp = nc.declare_dram_parameter
    xp = dp("xp", [128, KT, T], F16, isOutput=False)
    tp = dp("tp", [128, KT, T], F16, isOutput=False)
    wconv = dp("wconv", [128, KT, D], F16, isOutput=False)
    wqk = dp("wqk", [L, 128, KT, 2 * D], F16, isOutput=False)
    wv = dp("wv", [L, 128, KT, D], F16, isOutput=False)
    wo = dp("wo", [L, 128, KT, D], F16, isOutput=False)
    w1 = dp("w1", [L, 128, KT, MLPD], F16, isOutput=False)
    w2 = dp("w2", [L, 128, KM, D], F16, isOutput=False)
    whead = dp("whead", [128, KT, NCLS], F16, isOutput=False)
    qkb = dp("qkb", [L, 128, 12], F32, isOutput=False)
    vbrow = dp("vbrow", [L, 1, D], F16, isOutput=False)
    obrow = dp("obrow", [L, 1, D], F16, isOutput=False)
    f1b = dp("f1b", [L, 128, KM], F32, isOutput=False)
    f2row = dp("f2row", [L, 1, D], F16, isOutput=False)
    lng = {1: dp("g1", [L, 128, KT], F32, isOutput=False), 2: dp("g2", [L, 128, KT], F32, isOutput=False)}
    lnb = {1: dp("b1", [L, 128, KT], F32, isOutput=False), 2: dp("b2", [L, 128, KT], F32, isOutput=False)}
    lnfg = dp("lnfg", [128, KT], F32, isOutput=False)
    lnfb = dp("lnfb", [128, KT], F32, isOutput=False)
    headb = dp("headb", [128, 8], F32, isOutput=False)
    pos_plus = dp("pos_plus", [128, KT, T], F16, isOutput=False)
    out_d = dp("out", [NI, 1000], F32, isOutput=True)
    jvp_d = dp("out_jvp", [NI, 1000], F32, isOutput=True)

    with tile.TileContext(nc) as tc, ExitStack() as ctx:
        res = ctx.enter_context(tc.tile_pool(name="res", bufs=1))
        act = ctx.enter_context(tc.tile_pool(name="act", bufs=1))
        sh = ctx.enter_context(tc.tile_pool(name="sh", bufs=1))
        wpool = ctx.enter_context(tc.tile_pool(name="wp", bufs=2))
        small = ctx.enter_context(tc.tile_pool(name="sm", bufs=2))
        scr = ctx.enter_context(tc.tile_pool(name="scr", bufs=2))
        pbp = ctx.enter_context(tc.tile_pool(name="pbp", bufs=2))
        rows = ctx.enter_context(tc.tile_pool(name="rows", bufs=1))
        cst = ctx.enter_context(tc.tile_pool(name="cst", bufs=1))
        pm = ctx.enter_context(tc.tile_pool(name="pm", bufs=4, space="PSUM"))
        pbc = ctx.enter_context(tc.tile_pool(name="pbc", bufs=2, space="PSUM"))
        psc = ctx.enter_context(tc.tile_pool(name="psc", bufs=2, space="PSUM"))

        P_ = {"pm": lambda s_: pm.tile(s_, F32, tag="pm", name="pm"),
              "bc": lambda s_: pbc.tile(s_, F32, tag="bc", name="bc"),
              "sc": lambda s_: psc.tile(s_, F32, tag="sc", name="sc")}

        ones_r = cst.tile([1, TC0], F16, tag="ones_r", name="ones_r")
        nc.vector.memset(ones_r, 1.0)
        ones_c = cst.tile([128, 1], F16, tag="ones_c", name="ones_c")
        nc.vector.memset(ones_c, 1.0)
        ones_m = cst.tile([1, 128], F16, tag="ones_m", name="ones_m")
        nc.vector.memset(ones_m, 1.0)
        epst = cst.tile([1, 1], F32, tag="eps", name="eps")
        nc.vector.memset(epst, EPS)
        lnfg_t = cst.tile([128, KT], F32, tag="lnfg", name="lnfg")
        nc.sync.dma_start(out=lnfg_t, in_=lnfg[:, :])
        lnfb_t = cst.tile([128, KT], F32, tag="lnfb", name="lnfb")
        nc.sync.dma_start(out=lnfb_t, in_=lnfb[:, :])
        headb_t = cst.tile([128, 8], F32, tag="headb", name="headb")
        nc.sync.dma_start(out=headb_t, in_=headb[:, :])

        z16 = [res.tile([128, T], F16, tag=f"z{k}", name=f"z{k}") for k in range(KT)]
        tz16 = [res.tile([128, T], F16, tag=f"tz{k}", name=f"tz{k}") for k in range(KT)]
        y16 = [act.tile([128, T], F16, tag=f"y16_{k}", name=f"y16_{k}") for k in range(KT)]
        ty16 = [act.tile([128, T], F16, tag=f"ty16_{k}", name=f"ty16_{k}") for k in range(KT)]
        o16 = y16   # attention output reuses y16 (y consumed by qkv/v before)
        to16 = ty16

        def T1(shape, dt=F16):
            return sh.tile(shape, dt, tag="T1", name="T1")

        def T2(shape, dt=F16):
            return sh.tile(shape, dt, tag="T2", name="T2")

        def T3(shape, dt=F16):
            return sh.tile(shape, dt, tag="T3", name="T3")

        def T4(shape, dt=F16):
            return sh.tile(shape, dt, tag="T4", name="T4")

        def stt_v(out, in0, in1, op1, op0=OP.mult):
            nc.vector.scalar_tensor_tensor(out=out, in0=in0, scalar=1.0, in1=in1, op0=op0, op1=op1)

        # ---------------- patch embedding ----------------
        xp16 = T1([128, KT, T])
        tp16 = T2([128, KT, T])
        nc.sync.dma_start(out=xp16, in_=xp[:, :, :])
        nc.sync.dma_start(out=tp16, in_=tp[:, :, :])
        for mg in range(3):
            wt = wpool.tile([128, KT, 256], F16, tag="w", name="w")
            nc.sync.dma_start(out=wt, in_=wconv[:, :, mg * 256 : (mg + 1) * 256])
            for ms in range(2):
                m = 2 * mg + ms
                for ch in range(NCH):
                    sl = slice(ch * TC0, (ch + 1) * TC0)
                    ps_p = P_["pm"]([128, TC0])
                    ps_t = P_["pm"]([128, TC0])
                    for kt in range(KT):
                        nc.tensor.matmul(ps_p[:, :], wt[:, kt, ms * 128 : ms * 128 + 128], xp16[:, kt, sl],
                                         start=(kt == 0), stop=(kt == KT - 1))
                    for kt in range(KT):
                        nc.tensor.matmul(ps_t[:, :], wt[:, kt, ms * 128 : ms * 128 + 128], tp16[:, kt, sl],
                                         start=(kt == 0), stop=(kt == KT - 1))
                    nc.scalar.activation(out=z16[m][:, sl], in_=ps_p[:, :], func=AF.Identity)
                    nc.scalar.activation(out=tz16[m][:, sl], in_=ps_t[:, :], func=AF.Identity)
        for ch in range(NCH):
            sl = slice(ch * TC0, (ch + 1) * TC0)
            ppt = T3([128, KT, TC0])
            nc.sync.dma_start(out=ppt, in_=pos_plus[:, :, sl])
            for m in range(KT):
                stt_v(z16[m][:, sl], z16[m][:, sl], ppt[:, m, :], OP.add)

        # ---------------- layernorm: stats + 4x-mode apply ----------------
        def layernorm(which, l):
            gt = small.tile([128, KT], F32, tag="gt", name="gt")
            bt = small.tile([128, KT], F32, tag="bt", name="bt")
            nc.sync.dma_start(out=gt, in_=lng[which][l, :, :])
            nc.sync.dma_start(out=bt, in_=lnb[which][l, :, :])
            prs = [P_[p]([1, TC0]) for p in ("pm", "pm", "pm", "pm", "bc", "bc", "sc", "sc")]
            for si in range(4):
                for ch in range(NCH):
                    sl = slice(ch * TC0, (ch + 1) * TC0)
                    for k in range(KT):
                        if si == 0:
                            opnd = z16[k][:, sl]
                        elif si == 2:
                            opnd = tz16[k][:, sl]
                        else:
                            opnd = scr.tile([128, TC0], F16, tag="sq", name="sq")
                            stt_v(opnd, z16[k][:, sl],
                                  (z16[k][:, sl] if si == 1 else tz16[k][:, sl]), OP.mult)
                        nc.tensor.matmul(prs[2 * si + ch][0:1, :], ones_c, opnd,
                                         start=(k == 0), stop=(k == KT - 1))

            def row(tag, dt=F32):
                return rows.tile([1, T], dt, tag=tag, name=tag)

            mu, q2, tmu, cov = row("mu"), row("q2"), row("tmu"), row("cov")
            rstd = row("rstd")
            for ch in range(NCH):
                sl = slice(ch * TC0, (ch + 1) * TC0)
                nc.scalar.mul(out=mu[0:1, sl], in_=prs[0 + ch][0:1, :], mul=1.0 / D)
                nc.scalar.mul(out=q2[0:1, sl], in_=prs[2 + ch][0:1, :], mul=1.0 / D)
                nc.scalar.mul(out=tmu[0:1, sl], in_=prs[4 + ch][0:1, :], mul=1.0 / D)
                nc.scalar.mul(out=cov[0:1, sl], in_=prs[6 + ch][0:1, :], mul=1.0 / D)
            nc.vector.tensor_mul(out=rstd, in0=mu, in1=mu)
            nc.vector.tensor_sub(out=q2, in0=q2, in1=rstd)         # var
            nc.scalar.activation(out=q2, in_=q2, func=AF.Sqrt, bias=epst[0:1, 0:1])
            nc.vector.reciprocal(out=rstd, in_=q2)
            nc.vector.tensor_mul(out=q2, in0=mu, in1=tmu)
            nc.vector.tensor_sub(out=cov, in0=cov, in1=q2)         # cov
            rstd16 = row("rstd16", F16)
            mur16 = row("mur16", F16)
            tmur16 = row("tmur16", F16)
            cr16 = row("cr16", F16)
            nc.vector.tensor_copy(out=rstd16, in_=rstd)
            nc.vector.tensor_mul(out=mur16, in0=mu, in1=rstd)
            nc.vector.tensor_mul(out=tmur16, in0=tmu, in1=rstd)
            nc.vector.tensor_mul(out=q2, in0=rstd, in1=rstd)
            nc.vector.tensor_mul(out=cr16, in0=cov, in1=q2)
            for ch in range(NCH):
                c0 = ch * TC0
                sl = slice(c0, c0 + TC0)
                pbP = [P_["bc"]([128, TC0]), P_["sc"]([128, TC0]),
                       P_["bc"]([128, TC0]), P_["sc"]([128, TC0])]
                for j, r16 in enumerate((rstd16, mur16, tmur16, cr16)):
                    nc.tensor.matmul(pbP[j][:, :], ones_m, r16[0:1, sl], start=True, stop=True)
                pbA = pbp.tile([128, TC0], F16, tag="pbA", name="pbA", bufs=1)
                pbB = pbp.tile([128, TC0], F16, tag="pbB", name="pbB", bufs=1)
                pbC = pbp.tile([128, TC0], F16, tag="pbC", name="pbC", bufs=1)
                pbD = pbp.tile([128, TC0], F16, tag="pbD", name="pbD", bufs=1)
                for pb_t, pb_ps in zip((pbA, pbB, pbC, pbD), pbP):
                    nc.scalar.activation(out=pb_t, in_=pb_ps[:, :], func=AF.Identity)
                for k in range(KT):
                    yn = scr.tile([128, TC0], F16, tag="yn", name="yn")
                    a = scr.tile([128, TC0], F16, tag="a", name="a")
                    cx = scr.tile([128, TC0], F16, tag="cx", name="cx")
                    stt_v(yn, z16[k][:, sl], pbA, OP.mult)
                    stt_v(yn, yn, pbB, OP.subtract)
                    nc.vector.tensor_scalar(out=y16[k][:, sl], in0=yn, scalar1=gt[:, k : k + 1],
                                            scalar2=bt[:, k : k + 1], op0=OP.mult, op1=OP.add)
                    stt_v(a, tz16[k][:, sl], pbA, OP.mult)
                    stt_v(a, a, pbC, OP.subtract)
                    stt_v(cx, yn, pbD, OP.mult)
                    stt_v(a, a, cx, OP.subtract)
                    nc.vector.tensor_scalar(out=ty16[k][:, sl], in0=a, scalar1=gt[:, k : k + 1],
                                            scalar2=None, op0=OP.mult)

        # ---------------- layers ----------------
        for l in range(L_RUN):
            layernorm(1, l)
            qkbt = small.tile([128, 12], F32, tag="qkb", name="qkb")
            nc.sync.dma_start(out=qkbt, in_=qkb[l, :, :])
            vbr = small.tile([1, D], F16, tag="vbr", name="vbr")
            nc.sync.dma_start(out=vbr, in_=vbrow[l, :, :])
            # qk + v + attention, processed in image pairs
            wvt = wpool.tile([128, KT, D], F16, tag="wv", name="wv", bufs=1)
            nc.sync.dma_start(out=wvt, in_=wv[l, :, :, :])
            for pair in range(NI // 2):
              psl = slice(pair * TC0, (pair + 1) * TC0)
              qkiT = T3([128, 24, TC0])  # planes: q(0..5) k(6..11) tq(12..17) tk(18..23)
              for mg in range(6):
                wt = wpool.tile([128, KT, 256], F16, tag="w", name="w")
                nc.sync.dma_start(out=wt, in_=wqk[l, :, :, mg * 256 : (mg + 1) * 256])
                for ms in range(2):
                    m = 2 * mg + ms
                    ps_p = P_["pm"]([128, TC0])
                    ps_t = P_["pm"]([128, TC0])
                    for kt in range(KT):
                        nc.tensor.matmul(ps_p[:, :], wt[:, kt, ms * 128 : ms * 128 + 128],
                                         y16[kt][:, psl], start=(kt == 0), stop=(kt == KT - 1))
                    for kt in range(KT):
                        nc.tensor.matmul(ps_t[:, :], wt[:, kt, ms * 128 : ms * 128 + 128],
                                         ty16[kt][:, psl], start=(kt == 0), stop=(kt == KT - 1))
                    nc.scalar.activation(out=qkiT[:, m, :], in_=ps_p[:, :], func=AF.Identity,
                                         bias=qkbt[:, m : m + 1])
                    nc.scalar.activation(out=qkiT[:, 12 + m, :], in_=ps_t[:, :], func=AF.Identity)
              vti = [sh.tile([128, 4, D], F16, tag=f"T4_{j}", name=f"T4_{j}") for j in range(2)]
              for j in range(2):
                i0 = (2 * pair + j) * S
                for nchi in range(2):
                    for jvp in range(2):
                        src = y16 if jvp == 0 else ty16
                        for mc in range(2):
                            mr = 128 if mc == 0 else S - 128
                            t0 = i0 + mc * 128
                            pv = P_["pm"]([128, TC0])
                            for kt in range(KT):
                                nc.tensor.matmul(pv[0:mr, 0:384], src[kt][:, t0 : t0 + mr],
                                                 wvt[:, kt, nchi * 384 : (nchi + 1) * 384],
                                                 start=(kt == 0), stop=(kt == KT - 1 and jvp == 1))
                            if jvp == 0:
                                nc.tensor.matmul(pv[0:mr, 0:384], ones_m[0:1, 0:mr],
                                                 vbr[0:1, nchi * 384 : (nchi + 1) * 384], start=False, stop=True)
                            nc.scalar.activation(out=vti[j][0:mr, 2 * jvp + mc, nchi * 384 : (nchi + 1) * 384],
                                                 in_=pv[0:mr, 0:384], func=AF.Identity)

              for j in range(2):
                i0 = j * S                    # local offset within qkiT pair tile
                i0g = (2 * pair + j) * S      # global token offset
                ets = T1([128, 4, 12 * S])   # E c0, E c1, fh c0, fh c1
                att = T2([128, 2, 24 * S])   # [A(12S) | TA(12S)] per sk-chunk
                rrec = rows.tile([1, 12 * S], F16, tag="rrec", name="rrec")
                rr = rows.tile([1, 12 * S], F16, tag="rr", name="rr")
                for h in range(12):
                    qa = qkiT[(h % 2) * 64 : (h % 2) * 64 + 64, h // 2, i0 : i0 + S]
                    tqa = qkiT[(h % 2) * 64 : (h % 2) * 64 + 64, 12 + h // 2, i0 : i0 + S]
                    for c in range(2):
                        skn = 128 if c == 0 else S - 128
                        k0 = i0 + c * 128
                        ka = qkiT[(h % 2) * 64 : (h % 2) * 64 + 64, 6 + h // 2, k0 : k0 + skn]
                        tka = qkiT[(h % 2) * 64 : (h % 2) * 64 + 64, 18 + h // 2, k0 : k0 + skn]
                        pss = P_["sc"]([128, 2 * S])
                        rhs_qtq = bass.AP(tensor=qa.tensor, offset=qa.offset,
                                          ap=[list(qa.ap[0]), [12 * TC0, 2], [1, S]])
                        nc.tensor.matmul(pss[0:skn, 0 : 2 * S], ka, rhs_qtq, start=True, stop=False)
                        nc.tensor.matmul(pss[0:skn, S : 2 * S], tka, qa, start=False, stop=True)
                        nc.scalar.activation(out=ets[0:skn, c, h * S : (h + 1) * S], in_=pss[0:skn, 0:S],
                                             func=AF.Exp)
                        stt_v(ets[0:skn, 2 + c, h * S : (h + 1) * S],
                              ets[0:skn, c, h * S : (h + 1) * S], pss[0:skn, S : 2 * S], OP.mult)
                    pd = P_["bc"]([1, 2 * S])
                    for c in range(2):
                        skn = 128 if c == 0 else S - 128
                        nc.tensor.matmul(pd[0:1, 0:S], ones_c[0:skn, :], ets[0:skn, c, h * S : (h + 1) * S],
                                         start=(c == 0), stop=(c == 1))
                    for c in range(2):
                        skn = 128 if c == 0 else S - 128
                        nc.tensor.matmul(pd[0:1, S : 2 * S], ones_c[0:skn, :],
                                         ets[0:skn, 2 + c, h * S : (h + 1) * S],
                                         start=(c == 0), stop=(c == 1))
                    with nc.allow_low_precision(reason="softmax denom recip to fp16 is intentional"):
                        nc.vector.reciprocal(out=rrec[0:1, h * S : (h + 1) * S], in_=pd[0:1, 0:S])
                    nc.vector.tensor_tensor(out=rr[0:1, h * S : (h + 1) * S], in0=pd[0:1, S : 2 * S],
                                            in1=rrec[0:1, h * S : (h + 1) * S], op=OP.mult)
                for cc in range(6):
                    c0 = cc * TC0
                    csl = slice(c0, c0 + TC0)
                    pb1, pb2 = P_["bc"]([128, TC0]), P_["sc"]([128, TC0])
                    nc.tensor.matmul(pb1[:, :], ones_m, rrec[0:1, csl], start=True, stop=True)
                    nc.tensor.matmul(pb2[:, :], ones_m, rr[0:1, csl], start=True, stop=True)
                    pbE = pbp.tile([128, TC0], F16, tag="pbE", name="pbE")
                    pbF = pbp.tile([128, TC0], F16, tag="pbF", name="pbF")
                    nc.scalar.activation(out=pbE, in_=pb1[:, :], func=AF.Identity)
                    nc.scalar.activation(out=pbF, in_=pb2[:, :], func=AF.Identity)
                    for c in range(2):
                        t1 = scr.tile([128, TC0], F16, tag="t1", name="t1")
                        t2 = scr.tile([128, TC0], F16, tag="t2", name="t2")
                        stt_v(att[:, c, csl], ets[:, c, csl], pbE, OP.mult)          # A
                        stt_v(t1, ets[:, 2 + c, csl], pbE, OP.mult)                  # fh/denom
                        stt_v(t2, att[:, c, csl], pbF, OP.mult)                      # A*rr
                        stt_v(att[:, c, 12 * S + c0 : 12 * S + c0 + TC0], t1, t2, OP.subtract)
                for h in range(12):
                    po = P_["pm"]([64, 2 * S])
                    for c in range(2):
                        skn = 128 if c == 0 else S - 128
                        base = att[0:skn, c, h * S : h * S + S]
                        rhs2 = bass.AP(tensor=base.tensor, offset=base.offset,
                                       ap=[list(base.ap[0]), [12 * S, 2], [1, S]])
                        nc.tensor.matmul(po[0:64, 0 : 2 * S], vti[j][0:skn, c, h * 64 : (h + 1) * 64],
                                         rhs2, start=(c == 0), stop=False)
                    for c in range(2):
                        skn = 128 if c == 0 else S - 128
                        nc.tensor.matmul(po[0:64, S : 2 * S], vti[j][0:skn, 2 + c, h * 64 : (h + 1) * 64],
                                         att[0:skn, c, h * S : (h + 1) * S], start=False, stop=(c == 1))
                    nc.scalar.activation(out=o16[h // 2][(h % 2) * 64 : (h % 2) * 64 + 64, i0g : i0g + S],
                                         in_=po[0:64, 0:S], func=AF.Identity)
                    nc.vector.tensor_copy(out=to16[h // 2][(h % 2) * 64 : (h % 2) * 64 + 64, i0g : i0g + S],
                                          in_=po[0:64, S : 2 * S])

            # ---- output projection + residual ----
            obr = small.tile([1, D], F16, tag="obr", name="obr")
            nc.sync.dma_start(out=obr, in_=obrow[l, :, :])
            for mg in range(3):
                wt = wpool.tile([128, KT, 256], F16, tag="w", name="w")
                nc.sync.dma_start(out=wt, in_=wo[l, :, :, mg * 256 : (mg + 1) * 256])
                for ms in range(2):
                    m = 2 * mg + ms
                    for jvp in range(2):
                        src, tgt = (o16, z16) if jvp == 0 else (to16, tz16)
                        for ch in range(NCH):
                            sl = slice(ch * TC0, (ch + 1) * TC0)
                            pp_ = P_["pm"]([128, TC0])
                            for kt in range(KT):
                                nc.tensor.matmul(pp_[:, :], wt[:, kt, ms * 128 : ms * 128 + 128],
                                                 src[kt][:, sl], start=(kt == 0),
                                                 stop=(kt == KT - 1 and jvp == 1))
                            if jvp == 0:
                                nc.tensor.matmul(pp_[:, :], obr[0:1, m * 128 : (m + 1) * 128], ones_r,
                                                 start=False, stop=True)
                            if jvp == 0:
                                nc.vector.tensor_tensor(out=tgt[m][:, sl], in0=tgt[m][:, sl],
                                                        in1=pp_[:, :], op=OP.add)
                            else:
                                nc.vector.tensor_tensor(out=tgt[m][:, sl], in0=tgt[m][:, sl],
                                                        in1=pp_[:, :], op=OP.add)

            # ---- LN2 + MLP ----
            layernorm(2, l)
            f1bt = small.tile([128, KM], F32, tag="f1b", name="f1b")
            nc.sync.dma_start(out=f1bt, in_=f1b[l, :, :])
            f2r = small.tile([1, D], F16, tag="f2r", name="f2r")
            nc.sync.dma_start(out=f2r, in_=f2row[l, :, :])
            for half in range(NCH):
                c0 = half * TC0
                sl = slice(c0, c0 + TC0)
                h16 = T1([128, KM, TC0])
                th16 = T2([128, KM, TC0])
                for mg in range(12):
                    wt = wpool.tile([128, KT, 256], F16, tag="w", name="w")
                    nc.sync.dma_start(out=wt, in_=w1[l, :, :, mg * 256 : (mg + 1) * 256])
                    for ms in range(2):
                        m = 2 * mg + ms
                        pp_ = P_["pm"]([128, TC0])
                        pt_ = P_["pm"]([128, TC0])
                        for kt in range(KT):
                            nc.tensor.matmul(pp_[:, :], wt[:, kt, ms * 128 : ms * 128 + 128],
                                             y16[kt][:, sl], start=(kt == 0), stop=(kt == KT - 1))
                        for kt in range(KT):
                            nc.tensor.matmul(pt_[:, :], wt[:, kt, ms * 128 : ms * 128 + 128],
                                             ty16[kt][:, sl], start=(kt == 0), stop=(kt == KT - 1))
                        nc.scalar.activation(out=h16[:, m, :], in_=pp_[:, :], func=AF.Gelu,
                                             bias=f1bt[:, m : m + 1])
                        dg = scr.tile([128, TC0], F16, tag="dg", name="dg")
                        nc.scalar.activation(out=dg, in_=pp_[:, :], func=AF.Derivative_Gelu,
                                             bias=f1bt[:, m : m + 1])
                        nc.vector.tensor_tensor(out=th16[:, m, :], in0=dg, in1=pt_[:, :], op=OP.mult)
                for mg in range(3):
                    wt = wpool.tile([128, KM, 256], F16, tag="w2", name="w2")
                    nc.sync.dma_start(out=wt, in_=w2[l, :, :, mg * 256 : (mg + 1) * 256])
                    for ms in range(2):
                        m = 2 * mg + ms
                        pp_ = P_["pm"]([128, TC0])
                        pt_ = P_["pm"]([128, TC0])
                        for kt in range(KM):
                            nc.tensor.matmul(pp_[:, :], wt[:, kt, ms * 128 : ms * 128 + 128],
                                             h16[:, kt, :], start=(kt == 0), stop=False)
                        nc.tensor.matmul(pp_[:, :], f2r[0:1, m * 128 : (m + 1) * 128], ones_r,
                                         start=False, stop=True)
                        for kt in range(KM):
                            nc.tensor.matmul(pt_[:, :], wt[:, kt, ms * 128 : ms * 128 + 128],
                                             th16[:, kt, :], start=(kt == 0), stop=(kt == KM - 1))
                        nc.vector.tensor_tensor(out=z16[m][:, sl], in0=z16[m][:, sl], in1=pp_[:, :], op=OP.add)
                        nc.vector.tensor_tensor(out=tz16[m][:, sl], in0=tz16[m][:, sl], in1=pt_[:, :], op=OP.add)

        # ---------------- final LN + head ----------------
        zc = small.tile([128, KT, 8], F16, tag="zc", name="zc")
        sq8 = small.tile([128, KT, 8], F16, tag="sq8", name="sq8")
        for k in range(KT):
            ap_z = bass.AP(tensor=z16[k].tensor, offset=z16[k].offset, ap=[list(z16[k][:, 0:1].ap[0]), [S, NI]])
            ap_t = bass.AP(tensor=tz16[k].tensor, offset=tz16[k].offset, ap=[list(tz16[k][:, 0:1].ap[0]), [S, NI]])
            nc.vector.tensor_copy(out=zc[:, k, 0:4], in_=ap_z)
            nc.vector.tensor_copy(out=zc[:, k, 4:8], in_=ap_t)
            nc.vector.tensor_mul(out=sq8[:, k, 0:4], in0=zc[:, k, 0:4], in1=zc[:, k, 0:4])
            nc.vector.tensor_mul(out=sq8[:, k, 4:8], in0=zc[:, k, 0:4], in1=zc[:, k, 4:8])
        p8a = P_["bc"]([1, 8])
        p8b = P_["sc"]([1, 8])
        for k in range(KT):
            nc.tensor.matmul(p8a[0:1, :], ones_c, zc[:, k, :], start=(k == 0), stop=(k == KT - 1))
        for k in range(KT):
            nc.tensor.matmul(p8b[0:1, :], ones_c, sq8[:, k, :], start=(k == 0), stop=(k == KT - 1))
        r8 = rows.tile([1, 24], F32, tag="r8", name="r8")
        nc.scalar.mul(out=r8[0:1, 0:8], in_=p8a[0:1, :], mul=1.0 / D)
        nc.scalar.mul(out=r8[0:1, 8:16], in_=p8b[0:1, :], mul=1.0 / D)
        mu8, tmu8 = r8[0:1, 0:4], r8[0:1, 4:8]
        q28, cv8 = r8[0:1, 8:12], r8[0:1, 12:16]
        var8, rstd8 = r8[0:1, 16:20], r8[0:1, 20:24]
        nc.vector.tensor_mul(out=var8, in0=mu8, in1=mu8)
        nc.vector.tensor_sub(out=var8, in0=q28, in1=var8)
        nc.scalar.activation(out=var8, in_=var8, func=AF.Sqrt, bias=epst[0:1, 0:1])
        nc.vector.reciprocal(out=rstd8, in_=var8)
        r8b = rows.tile([1, 24], F16, tag="r8b", name="r8b")
        nc.vector.tensor_copy(out=r8b[0:1, 0:4], in_=rstd8)
        nc.vector.tensor_copy(out=r8b[0:1, 4:8], in_=rstd8)
        nc.vector.tensor_mul(out=r8b[0:1, 8:12], in0=mu8, in1=rstd8)
        nc.vector.tensor_mul(out=q28, in0=mu8, in1=tmu8)
        nc.vector.tensor_sub(out=cv8, in0=cv8, in1=q28)
        nc.vector.tensor_mul(out=r8b[0:1, 12:16], in0=tmu8, in1=rstd8)
        nc.vector.tensor_mul(out=q28, in0=rstd8, in1=rstd8)
        nc.vector.tensor_mul(out=cv8, in0=cv8, in1=q28)
        nc.vector.tensor_copy(out=r8b[0:1, 16:20], in_=cv8)
        pbA = P_["bc"]([128, 8])
        pbB = P_["sc"]([128, 8])
        pbG = P_["bc"]([128, 4])
        nc.tensor.matmul(pbA[:, :], ones_m, r8b[0:1, 0:8], start=True, stop=True)
        nc.tensor.matmul(pbB[:, :], ones_m, r8b[0:1, 8:16], start=True, stop=True)
        nc.tensor.matmul(pbG[:, :], ones_m, r8b[0:1, 16:20], start=True, stop=True)
        y0 = small.tile([128, KT, 8], F16, tag="y0", name="y0")
        for k in range(KT):
            w0 = small.tile([128, 8], F16, tag="w0", name="w0")
            tg8 = small.tile([128, 4], F16, tag="tg8", name="tg8")
            nc.vector.tensor_tensor(out=w0, in0=zc[:, k, :], in1=pbA[:, :], op=OP.mult)
            nc.vector.tensor_tensor(out=w0, in0=w0, in1=pbB[:, :], op=OP.subtract)
            nc.vector.tensor_tensor(out=tg8, in0=w0[:, 0:4], in1=pbG[:, :], op=OP.mult)
            nc.vector.tensor_sub(out=w0[:, 4:8], in0=w0[:, 4:8], in1=tg8)
            nc.vector.tensor_scalar(out=y0[:, k, 0:4], in0=w0[:, 0:4], scalar1=lnfg_t[:, k : k + 1],
                                    scalar2=lnfb_t[:, k : k + 1], op0=OP.mult, op1=OP.add)
            nc.vector.tensor_scalar(out=y0[:, k, 4:8], in0=w0[:, 4:8], scalar1=lnfg_t[:, k : k + 1],
                                    scalar2=None, op0=OP.mult)
        for m in range(8):
            wt = wpool.tile([128, KT, 125], F16, tag="wh", name="wh")
            nc.sync.dma_start(out=wt, in_=whead[:, :, m * 125 : (m + 1) * 125])
            ph = P_["pm"]([125, 8])
            for kt in range(KT):
                nc.tensor.matmul(ph[0:125, :], wt[:, kt, :], y0[:, kt, :], start=(kt == 0), stop=(kt == KT - 1))
            lg = small.tile([128, 8], F32, tag="lg", name="lg")
            nc.vector.tensor_scalar(out=lg[0:125, 0:4], in0=ph[0:125, 0:4],
                                    scalar1=headb_t[:125, m : m + 1], scalar2=None, op0=OP.add)
            nc.vector.tensor_copy(out=lg[0:125, 4:8], in_=ph[0:125, 4:8])
            o_ap = bass.AP(tensor=out_d, offset=m * 125, ap=[[1, 125], [1000, NI]])
            j_ap = bass.AP(tensor=jvp_d, offset=m * 125, ap=[[1, 125], [1000, NI]])
            nc.sync.dma_start(out=o_ap, in_=lg[0:125, 0:4])
            nc.sync.dma_start(out=j_ap, in_=lg[0:125, 4:8])

    nc.compile()
    return nc


_PROG = None


def kernel(**inputs):
    global _PROG
    from concourse.bass_utils import run_bass_kernel_spmd

    inputs = {k: np.asarray(v) for k, v in inputs.items()}
    host = prep_host(inputs)
    if _PROG is None:
        _PROG = build_program()
    in_maps = []
    for c in range(NCORES):
        m = dict(host)
        m["xp"] = prep_patches(np.asarray(inputs["x"][c * NI : (c + 1) * NI], np.float32))
        m["tp"] = prep_patches(np.asarray(inputs["tangent"][c * NI : (c + 1) * NI], np.float32))
        in_maps.append(m)
    res_ = run_bass_kernel_spmd(_PROG, in_maps, list(range(NCORES)))
    out = np.concatenate([res_.results[c]["out"] for c in range(NCORES)], axis=0)
    jvp = np.concatenate([res_.results[c]["out_jvp"] for c in range(NCORES)], axis=0)
    return out.astype(np.float32), jvp.astype(np.float32)
